# revision 20
# baseline (speedup 1.0000x reference)
"""Trainium2 Bass kernel for nn_AntiSymmetric GNN message passing.

Strategy (8 NeuronCores, SPMD, no collectives):
  - Nodes sharded by destination: core c owns dst rows [c*12500, (c+1)*12500).
  - Edges partitioned by dst owner on host, grouped per 128-dst tile and per
    source bank (4 banks of 25000 table rows; dma_gather indices are int16),
    padded to an SPMD-uniform chunk count K0[tile][bank] (max over cores) so
    all 8 cores run one compiled schedule.
  - Full embedding table replicated to every core's HBM (host staging).
  - Per core on device: dma_gather source rows (256B f32 rows) in 2048-edge
    calls into SBUF; convert to bf16; segment-sum on the TensorEngine:
    for each 128-edge chunk, matmul(lhsT=G[128 slots, 64 feat],
    rhs=onehot[128 slots, 128 dsts]) accumulates aggT[64, 128] in PSUM.
    The one-hot is built on the VectorEngine from a per-slot local-dst
    tensor vs a 0..127 ramp (is_equal, bf16); padded slots carry dst=-1 and
    contribute zero. Duplicate dsts are plain PSUM adds (the dma_scatter_add
    CCE path loses duplicate-row updates in flight, measured on HW).
  - Fused per-tile epilogue in feature-major f32: h = W_rel.T@aggT +
    W_comb.T@xT (+b_comb via ACT tanh), x' = x + 0.1*h,
    out = sigmoid(W_lin.T@x' + b_lin), one final [16, 12544] DMA per core.
  - Host transposes/concats the per-core outputs.

Weight preprocessing (host): aW = W_anti - W_anti.T - 0.1*I,
W_comb = W_root + aW, b_comb = b_rel + b_anti.
"""

import os

os.environ.setdefault("NEURON_RT_RESET_CORES", "1")

import numpy as np

import concourse.bass as bass
import concourse.mybir as mybir
from concourse import bacc
import concourse.tile as tile
from concourse import library_config
from concourse.bass_utils import run_bass_kernel_spmd

F32 = mybir.dt.float32
BF16 = mybir.dt.bfloat16
I16 = mybir.dt.int16


def _cfg_full():
    return dict(
        N=100000, E=1600000, D=64, C=16,
        NCORES=8, NPC=12500, NPC_PAD=12544,
        NBANK=4, BANKR=25000, CH=4096, TILE=256,
    )


def _prep_edges(edge_index, cfg):
    """Group edges per (core, dst-tile, bank); build the SPMD-uniform
    schedule plus per-core gather-index and local-dst tensors."""
    NCORES, NPC, NPC_PAD = cfg["NCORES"], cfg["NPC"], cfg["NPC_PAD"]
    NBANK, BANKR, CH = cfg["NBANK"], cfg["BANKR"], cfg["CH"]
    TILE = cfg["TILE"]
    NT = NPC_PAD // TILE

    src = np.asarray(edge_index[0], dtype=np.int64)
    dst = np.asarray(edge_index[1], dtype=np.int64)
    owner = dst // NPC

    per_core = []
    n_ctb = np.zeros((NCORES, NT, NBANK), dtype=np.int64)
    for c in range(NCORES):
        m = owner == c
        s = src[m]
        dl = dst[m] - c * NPC
        t = dl // TILE
        b = s // BANKR
        o = np.lexsort((b, t))
        s, dl = s[o], dl[o]
        key = (dl // TILE) * NBANK + (s // BANKR)
        n_ctb[c] = np.bincount(key, minlength=NT * NBANK).reshape(NT, NBANK)
        per_core.append((s, dl))

    # SPMD-uniform chunks per (tile, bank)
    K0 = np.ceil(n_ctb.max(axis=0) / 128.0).astype(np.int64)  # [NT, NBANK]
    NCH = int(K0.sum())
    TOT = 128 * NCH
    SB = 128 * K0.sum(axis=0)                             # slots per bank
    SBpre = np.concatenate([[0], np.cumsum(SB)])[:NBANK]  # slot prefix
    posK = np.zeros((NT, NBANK), dtype=np.int64)          # chunk pos in bank
    posK[1:] = np.cumsum(K0, axis=0)[:-1]
    chbase = np.concatenate([[0], np.cumsum(K0.sum(axis=1))])  # per tile
    chb_tb = chbase[:-1][:, None] + np.concatenate(
        [np.zeros((NT, 1), np.int64), np.cumsum(K0, axis=1)[:, :-1]], axis=1)

    # per-tile consumption list: (bank, chunk-pos-in-bank-stream, global ch)
    chunk_list = []
    for t in range(NT):
        lst = []
        for b in range(NBANK):
            for j in range(int(K0[t, b])):
                lst.append((b, int(posK[t, b]) + j, int(chb_tb[t, b]) + j))
        chunk_list.append(lst)

    # per-bank gather calls: list of (col_off, cnt)
    calls = []
    for b in range(NBANK):
        cl = []
        off = 0
        while off < SB[b]:
            cnt = int(min(CH, SB[b] - off))
            cl.append((int(SBpre[b] + off) // 16, cnt))
            off += cnt
        calls.append(cl)

    gidx_list, dlt_list = [], []
    for c in range(NCORES):
        s, dl = per_core[c]
        gflat = np.zeros(TOT, dtype=np.int16)
        dlt = np.full((128, max(NCH, 1)), -1.0, dtype=np.float32)
        grp_off = np.zeros(NT * NBANK, dtype=np.int64)
        grp_off[1:] = np.cumsum(n_ctb[c].reshape(-1))[:-1]
        for t in range(NT):
            for b in range(NBANK):
                k0 = int(K0[t, b])
                if k0 == 0:
                    continue
                n = int(n_ctb[c, t, b])
                e0 = int(grp_off[t * NBANK + b])
                slot0 = int(SBpre[b]) + 128 * int(posK[t, b])
                gflat[slot0:slot0 + n] = (
                    s[e0:e0 + n] - b * BANKR).astype(np.int16)
                dpad = np.full(128 * k0, -1.0, np.float32)
                dpad[:n] = (dl[e0:e0 + n] - TILE * t).astype(np.float32)
                ch0 = int(chb_tb[t, b])
                dlt[:, ch0:ch0 + k0] = dpad.reshape(k0, 128).T
        # wrap gather idxs: slot i -> [i % 16, i // 16]; tile to 128 parts
        gidx_list.append(np.tile(gflat.reshape(-1, 16).T, (8, 1)).copy())
        dlt_list.append(dlt)

    sched = dict(K0=K0, NCH=NCH, TOT=TOT, SB=SB, SBpre=SBpre,
                 chunk_list=chunk_list, calls=calls, NT=NT)
    return sched, gidx_list, dlt_list


def _build(cfg, sched):
    N, D, C = cfg["N"], cfg["D"], cfg["C"]
    NPC_PAD, TILE = cfg["NPC_PAD"], cfg["TILE"]
    NBANK, BANKR, CH = cfg["NBANK"], cfg["BANKR"], cfg["CH"]
    NT, NCH, TOT = sched["NT"], sched["NCH"], sched["TOT"]
    chunk_list, calls = sched["chunk_list"], sched["calls"]

    nc = bacc.Bacc("TRN2")
    xtab = nc.declare_dram_parameter("xtab", [N, D], F32, isOutput=False)
    gidx = nc.declare_dram_parameter("gidx", [128, TOT // 16], I16,
                                     isOutput=False)
    dltp = nc.declare_dram_parameter("dlt", [128, max(NCH, 1)], F32,
                                     isOutput=False)
    xTp = nc.declare_dram_parameter("xT", [D, NPC_PAD], F32, isOutput=False)
    wrelT = nc.declare_dram_parameter("wrelT", [D, D], F32, isOutput=False)
    wcombT = nc.declare_dram_parameter("wcombT", [D, D], F32, isOutput=False)
    wlinT = nc.declare_dram_parameter("wlinT", [D, C], F32, isOutput=False)
    bcomb = nc.declare_dram_parameter("bcomb", [D, 1], F32, isOutput=False)
    blin = nc.declare_dram_parameter("blin", [C, 1], F32, isOutput=False)
    rampp = nc.declare_dram_parameter("ramp", [128, TILE], F32, isOutput=False)
    outT = nc.declare_dram_parameter("outT", [C, NPC_PAD], F32, isOutput=True)

    AF = mybir.ActivationFunctionType
    OP = mybir.AluOpType

    with tile.TileContext(nc) as tc:
        with (
            tc.tile_pool(name="const", bufs=1) as cpool,
            tc.tile_pool(name="gath", bufs=2) as gpool,
            tc.tile_pool(name="oh", bufs=3) as opool,
            tc.tile_pool(name="xt", bufs=2) as xpool,
            tc.tile_pool(name="ep", bufs=3) as epool,
            tc.tile_pool(name="psum", bufs=2, space="PSUM") as ppool,
        ):
            nc.gpsimd.load_library(library_config.mlp)

            t_gidx = cpool.tile([128, TOT // 16], I16)
            t_dlt = cpool.tile([128, max(NCH, 1)], F32)
            t_wrelT = cpool.tile([D, D], F32)
            t_wcombT = cpool.tile([D, D], F32)
            t_wlinT = cpool.tile([D, C], F32)
            t_bcomb = cpool.tile([D, 1], F32)
            t_blin = cpool.tile([C, 1], F32)
            t_ramp = cpool.tile([128, TILE], F32)

            nc.sync.dma_start(t_gidx[:], gidx[:])
            nc.sync.dma_start(t_dlt[:], dltp[:])
            nc.sync.dma_start(t_wrelT[:], wrelT[:])
            nc.sync.dma_start(t_wcombT[:], wcombT[:])
            nc.sync.dma_start(t_wlinT[:], wlinT[:])
            nc.sync.dma_start(t_bcomb[:], bcomb[:])
            nc.sync.dma_start(t_blin[:], blin[:])
            nc.sync.dma_start(t_ramp[:], rampp[:])

            issued = [0] * NBANK   # gather calls issued per bank
            btiles = {}            # (bank, call) -> bf16 tile
            ohtiles = {}           # block -> (tile, width)

            def issue_call(b):
                k = issued[b]
                coff, cnt = calls[b][k]
                nch = cnt // 128
                gt = gpool.tile([128, nch, D], F32, tag=f"g{b}")
                nc.gpsimd.dma_gather(
                    gt[:], xtab[b * BANKR:(b + 1) * BANKR, :],
                    t_gidx[:, coff:coff + cnt // 16], cnt, cnt, D,
                    single_packet=False)
                bt = gpool.tile([128, nch, D], BF16, tag=f"gb{b}")
                nc.vector.tensor_copy(bt[:], gt[:])
                btiles[(b, k)] = bt
                issued[b] = k + 1

            def get_oh(blk):
                if blk in ohtiles:
                    return ohtiles[blk]
                w = min(16, NCH - blk * 16)
                oht = opool.tile([128, w, TILE], BF16, tag="oh")
                rb = t_ramp[:].unsqueeze(1).broadcast_to((128, w, TILE))
                db = t_dlt[:, blk * 16:blk * 16 + w].unsqueeze(2).broadcast_to(
                    (128, w, TILE))
                nc.vector.tensor_tensor(oht[:], rb, db, op=OP.is_equal)
                ohtiles[blk] = (oht, w)
                return ohtiles[blk]

            xblk = None
            XB = 8 * 128 // TILE  # tiles per xT block (1024 dsts)
            for t in range(NT):
                lst = chunk_list[t]
                aggT = epool.tile([D, TILE], F32, tag="aggT")
                if lst:
                    pagg = ppool.tile([D, TILE], F32, tag="pagg")
                    nlst = len(lst)
                    for i, (b, pos, ch) in enumerate(lst):
                        call_i = pos // (CH // 128)
                        slot = pos % (CH // 128)
                        while issued[b] <= call_i:
                            issue_call(b)
                        oht, _w = get_oh(ch // 16)
                        nc.tensor.matmul(
                            pagg[:], btiles[(b, call_i)][:, slot, :],
                            oht[:, ch % 16, :],
                            start=(i == 0), stop=(i == nlst - 1))
                    nc.vector.tensor_copy(aggT[:], pagg[:])
                else:
                    nc.vector.memset(aggT[:], 0.0)

                if t % XB == 0:
                    w = min(XB, NT - t) * TILE
                    xblk = xpool.tile([D, w], F32, tag="xT")
                    nc.sync.dma_start(xblk[:],
                                      xTp[:, t * TILE:t * TILE + w])
                xsl = xblk[:, (t % XB) * TILE:(t % XB + 1) * TILE]

                p_h = ppool.tile([D, TILE], F32, tag="ph")
                nc.tensor.matmul(p_h[:], t_wrelT[:], aggT[:],
                                 start=True, stop=False)
                nc.tensor.matmul(p_h[:], t_wcombT[:], xsl,
                                 start=False, stop=True)
                hT = epool.tile([D, TILE], F32, tag="hT")
                nc.scalar.activation(hT[:], p_h[:], AF.Tanh,
                                     bias=t_bcomb[:], scale=1.0)
                xnT = epool.tile([D, TILE], F32, tag="xnT")
                nc.vector.scalar_tensor_tensor(
                    xnT[:], hT[:], 0.1, xsl, op0=OP.mult, op1=OP.add)
                p_o = ppool.tile([C, TILE], F32, tag="po")
                nc.tensor.matmul(p_o[:], t_wlinT[:], xnT[:],
                                 start=True, stop=True)
                ot = epool.tile([C, TILE], F32, tag="ot")
                nc.scalar.activation(ot[:], p_o[:], AF.Sigmoid,
                                     bias=t_blin[:], scale=1.0)
                nc.sync.dma_start(outT[:, t * TILE:(t + 1) * TILE], ot[:])

    nc.compile()
    return nc


def _host_prep_weights(W_rel, b_rel, W_root, W_anti, b_anti, W_lin, b_lin, D):
    aW = W_anti - W_anti.T - 0.1 * np.eye(D, dtype=np.float32)
    W_comb = W_root + aW
    wrelT = np.ascontiguousarray(W_rel.T.astype(np.float32))
    wcombT = np.ascontiguousarray(W_comb.T.astype(np.float32))
    wlinT = np.ascontiguousarray(W_lin.T.astype(np.float32))
    bcomb = (b_rel + b_anti).astype(np.float32).reshape(-1, 1)
    blin = b_lin.astype(np.float32).reshape(-1, 1)
    return wrelT, wcombT, wlinT, bcomb, blin


TRACE = False
LAST_RESULTS = None
_BUILD_CACHE = {}


def _run(inputs, cfg):
    global LAST_RESULTS
    NCORES, NPC, NPC_PAD, D = (cfg["NCORES"], cfg["NPC"], cfg["NPC_PAD"],
                               cfg["D"])

    edge_index = np.asarray(inputs["edge_index"], dtype=np.int32)
    x = np.asarray(inputs["embed_w"], dtype=np.float32)

    sched, gidx_list, dlt_list = _prep_edges(edge_index, cfg)

    key = (sched["TOT"], sched["NCH"],
           tuple(np.asarray(sched["K0"]).reshape(-1).tolist()),
           tuple(sorted(cfg.items())))
    if key not in _BUILD_CACHE:
        _BUILD_CACHE[key] = _build(cfg, sched)
    nc = _BUILD_CACHE[key]

    wrelT, wcombT, wlinT, bcomb, blin = _host_prep_weights(
        np.asarray(inputs["W_rel"], np.float32),
        np.asarray(inputs["b_rel"], np.float32),
        np.asarray(inputs["W_root"], np.float32),
        np.asarray(inputs["W_anti"], np.float32),
        np.asarray(inputs["b_anti"], np.float32),
        np.asarray(inputs["W_lin"], np.float32),
        np.asarray(inputs["b_lin"], np.float32),
        D)
    ramp = np.tile(np.arange(cfg["TILE"], dtype=np.float32), (128, 1))

    in_maps = []
    for c in range(NCORES):
        xTc = np.zeros((D, NPC_PAD), dtype=np.float32)
        xTc[:, :NPC] = x[c * NPC:(c + 1) * NPC].T
        in_maps.append({
            "xtab": x,
            "gidx": gidx_list[c],
            "dlt": dlt_list[c],
            "xT": xTc,
            "wrelT": wrelT,
            "wcombT": wcombT,
            "wlinT": wlinT,
            "bcomb": bcomb,
            "blin": blin,
            "ramp": ramp,
        })

    res = run_bass_kernel_spmd(nc, in_maps, list(range(NCORES)), trace=TRACE)
    LAST_RESULTS = res
    out = np.concatenate(
        [np.asarray(res.results[c]["outT"]).T[:NPC] for c in range(NCORES)],
        axis=0).astype(np.float32)
    return out


def kernel(**inputs) -> np.ndarray:
    return _run(inputs, _cfg_full())


# revision 22
# speedup vs baseline: 1.2356x; 1.2356x over previous
"""Trainium2 Bass kernel for nn_AntiSymmetric GNN message passing.

Strategy (8 NeuronCores, SPMD, no collectives):
  - Nodes sharded by destination: core c owns dst rows [c*12500, (c+1)*12500).
  - Edges partitioned by dst owner on host, grouped per 128-dst tile and per
    source bank (4 banks of 25000 table rows; dma_gather indices are int16),
    padded to an SPMD-uniform chunk count K0[tile][bank] (max over cores) so
    all 8 cores run one compiled schedule.
  - Full embedding table replicated to every core's HBM (host staging).
  - Per core on device: dma_gather source rows (256B f32 rows) in 2048-edge
    calls into SBUF; convert to bf16; segment-sum on the TensorEngine:
    for each 128-edge chunk, matmul(lhsT=G[128 slots, 64 feat],
    rhs=onehot[128 slots, 128 dsts]) accumulates aggT[64, 128] in PSUM.
    The one-hot is built on the VectorEngine from a per-slot local-dst
    tensor vs a 0..127 ramp (is_equal, bf16); padded slots carry dst=-1 and
    contribute zero. Duplicate dsts are plain PSUM adds (the dma_scatter_add
    CCE path loses duplicate-row updates in flight, measured on HW).
  - Fused per-tile epilogue in feature-major f32: h = W_rel.T@aggT +
    W_comb.T@xT (+b_comb via ACT tanh), x' = x + 0.1*h,
    out = sigmoid(W_lin.T@x' + b_lin), one final [16, 12544] DMA per core.
  - Host transposes/concats the per-core outputs.

Weight preprocessing (host): aW = W_anti - W_anti.T - 0.1*I,
W_comb = W_root + aW, b_comb = b_rel + b_anti.
"""

import os

os.environ.setdefault("NEURON_RT_RESET_CORES", "1")

import numpy as np

import concourse.bass as bass
import concourse.mybir as mybir
from concourse import bacc
import concourse.tile as tile
from concourse import library_config
from concourse.bass_utils import run_bass_kernel_spmd

F32 = mybir.dt.float32
BF16 = mybir.dt.bfloat16
I16 = mybir.dt.int16


def _cfg_full():
    return dict(
        N=100000, E=1600000, D=64, C=16,
        NCORES=8, NPC=12500, NPC_PAD=12544,
        NBANK=4, BANKR=25000, CH=2048, TILE=256,
    )


def _prep_edges(edge_index, cfg):
    """Group edges per (core, dst-tile, bank); build the SPMD-uniform
    schedule plus per-core gather-index and local-dst tensors."""
    NCORES, NPC, NPC_PAD = cfg["NCORES"], cfg["NPC"], cfg["NPC_PAD"]
    NBANK, BANKR, CH = cfg["NBANK"], cfg["BANKR"], cfg["CH"]
    TILE = cfg["TILE"]
    NT = NPC_PAD // TILE

    src = np.asarray(edge_index[0], dtype=np.int64)
    dst = np.asarray(edge_index[1], dtype=np.int64)
    owner = dst // NPC

    per_core = []
    n_ctb = np.zeros((NCORES, NT, NBANK), dtype=np.int64)
    for c in range(NCORES):
        m = owner == c
        s = src[m]
        dl = dst[m] - c * NPC
        t = dl // TILE
        b = s // BANKR
        o = np.lexsort((b, t))
        s, dl = s[o], dl[o]
        key = (dl // TILE) * NBANK + (s // BANKR)
        n_ctb[c] = np.bincount(key, minlength=NT * NBANK).reshape(NT, NBANK)
        per_core.append((s, dl))

    # SPMD-uniform chunks per (tile, bank)
    K0 = np.ceil(n_ctb.max(axis=0) / 128.0).astype(np.int64)  # [NT, NBANK]
    NCH = int(K0.sum())
    TOT = 128 * NCH
    SB = 128 * K0.sum(axis=0)                             # slots per bank
    SBpre = np.concatenate([[0], np.cumsum(SB)])[:NBANK]  # slot prefix
    posK = np.zeros((NT, NBANK), dtype=np.int64)          # chunk pos in bank
    posK[1:] = np.cumsum(K0, axis=0)[:-1]
    chbase = np.concatenate([[0], np.cumsum(K0.sum(axis=1))])  # per tile
    chb_tb = chbase[:-1][:, None] + np.concatenate(
        [np.zeros((NT, 1), np.int64), np.cumsum(K0, axis=1)[:, :-1]], axis=1)

    # per-tile consumption list: (bank, chunk-pos-in-bank-stream, global ch)
    chunk_list = []
    for t in range(NT):
        lst = []
        for b in range(NBANK):
            for j in range(int(K0[t, b])):
                lst.append((b, int(posK[t, b]) + j, int(chb_tb[t, b]) + j))
        chunk_list.append(lst)

    # per-bank gather calls: list of (col_off, cnt)
    calls = []
    for b in range(NBANK):
        cl = []
        off = 0
        while off < SB[b]:
            cnt = int(min(CH, SB[b] - off))
            cl.append((int(SBpre[b] + off) // 16, cnt))
            off += cnt
        calls.append(cl)

    gidx_list, dlt_list = [], []
    for c in range(NCORES):
        s, dl = per_core[c]
        gflat = np.zeros(TOT, dtype=np.int16)
        dlt = np.full((128, max(NCH, 1)), -1.0, dtype=np.float32)
        grp_off = np.zeros(NT * NBANK, dtype=np.int64)
        grp_off[1:] = np.cumsum(n_ctb[c].reshape(-1))[:-1]
        for t in range(NT):
            for b in range(NBANK):
                k0 = int(K0[t, b])
                if k0 == 0:
                    continue
                n = int(n_ctb[c, t, b])
                e0 = int(grp_off[t * NBANK + b])
                slot0 = int(SBpre[b]) + 128 * int(posK[t, b])
                gflat[slot0:slot0 + n] = (
                    s[e0:e0 + n] - b * BANKR).astype(np.int16)
                dpad = np.full(128 * k0, -1.0, np.float32)
                dpad[:n] = (dl[e0:e0 + n] - TILE * t).astype(np.float32)
                ch0 = int(chb_tb[t, b])
                dlt[:, ch0:ch0 + k0] = dpad.reshape(k0, 128).T
        # wrap gather idxs: slot i -> [i % 16, i // 16]; tile to 128 parts
        gidx_list.append(np.tile(gflat.reshape(-1, 16).T, (8, 1)).copy())
        dlt_list.append(dlt)

    sched = dict(K0=K0, NCH=NCH, TOT=TOT, SB=SB, SBpre=SBpre,
                 chunk_list=chunk_list, calls=calls, NT=NT)
    return sched, gidx_list, dlt_list


def _build(cfg, sched):
    N, D, C = cfg["N"], cfg["D"], cfg["C"]
    NPC_PAD, TILE = cfg["NPC_PAD"], cfg["TILE"]
    NBANK, BANKR, CH = cfg["NBANK"], cfg["BANKR"], cfg["CH"]
    NT, NCH, TOT = sched["NT"], sched["NCH"], sched["TOT"]
    chunk_list, calls = sched["chunk_list"], sched["calls"]

    nc = bacc.Bacc("TRN2")
    xtab = nc.declare_dram_parameter("xtab", [N, D], F32, isOutput=False)
    gidx = nc.declare_dram_parameter("gidx", [128, TOT // 16], I16,
                                     isOutput=False)
    dltp = nc.declare_dram_parameter("dlt", [128, max(NCH, 1)], F32,
                                     isOutput=False)
    xTp = nc.declare_dram_parameter("xT", [D, NPC_PAD], F32, isOutput=False)
    wrelT = nc.declare_dram_parameter("wrelT", [D, D], F32, isOutput=False)
    wcombT = nc.declare_dram_parameter("wcombT", [D, D], F32, isOutput=False)
    wlinT = nc.declare_dram_parameter("wlinT", [D, C], F32, isOutput=False)
    bcomb = nc.declare_dram_parameter("bcomb", [D, 1], F32, isOutput=False)
    blin = nc.declare_dram_parameter("blin", [C, 1], F32, isOutput=False)
    rampp = nc.declare_dram_parameter("ramp", [128, TILE], F32, isOutput=False)
    outT = nc.declare_dram_parameter("outT", [C, NPC_PAD], F32, isOutput=True)

    AF = mybir.ActivationFunctionType
    OP = mybir.AluOpType

    with tile.TileContext(nc) as tc:
        with (
            tc.tile_pool(name="const", bufs=1) as cpool,
            tc.tile_pool(name="gath", bufs=3) as gpool,
            tc.tile_pool(name="oh", bufs=3) as opool,
            tc.tile_pool(name="xt", bufs=2) as xpool,
            tc.tile_pool(name="ep", bufs=3) as epool,
            tc.tile_pool(name="psum", bufs=2, space="PSUM") as ppool,
        ):
            nc.gpsimd.load_library(library_config.mlp)

            t_gidx = cpool.tile([128, TOT // 16], I16)
            t_dlt = cpool.tile([128, max(NCH, 1)], F32)
            t_wrelT = cpool.tile([D, D], F32)
            t_wcombT = cpool.tile([D, D], F32)
            t_wlinT = cpool.tile([D, C], F32)
            t_bcomb = cpool.tile([D, 1], F32)
            t_blin = cpool.tile([C, 1], F32)
            t_ramp = cpool.tile([128, TILE], F32)

            nc.sync.dma_start(t_gidx[:], gidx[:])
            nc.sync.dma_start(t_dlt[:], dltp[:])
            nc.sync.dma_start(t_wrelT[:], wrelT[:])
            nc.sync.dma_start(t_wcombT[:], wcombT[:])
            nc.sync.dma_start(t_wlinT[:], wlinT[:])
            nc.sync.dma_start(t_bcomb[:], bcomb[:])
            nc.sync.dma_start(t_blin[:], blin[:])
            nc.sync.dma_start(t_ramp[:], rampp[:])

            issued = [0] * NBANK   # gather calls issued per bank
            btiles = {}            # (bank, call) -> bf16 tile
            ohtiles = {}           # block -> (tile, width)

            def issue_call(b):
                k = issued[b]
                coff, cnt = calls[b][k]
                nch = cnt // 128
                gt = gpool.tile([128, nch, D], F32, tag=f"g{b}")
                nc.gpsimd.dma_gather(
                    gt[:], xtab[b * BANKR:(b + 1) * BANKR, :],
                    t_gidx[:, coff:coff + cnt // 16], cnt, cnt, D,
                    single_packet=False)
                bt = gpool.tile([128, nch, D], BF16, tag=f"gb{b}")
                nc.vector.tensor_copy(bt[:], gt[:])
                btiles[(b, k)] = bt
                issued[b] = k + 1

            def get_oh(blk):
                if blk in ohtiles:
                    return ohtiles[blk]
                w = min(16, NCH - blk * 16)
                oht = opool.tile([128, w, TILE], BF16, tag="oh")
                rb = t_ramp[:].unsqueeze(1).broadcast_to((128, w, TILE))
                db = t_dlt[:, blk * 16:blk * 16 + w].unsqueeze(2).broadcast_to(
                    (128, w, TILE))
                nc.vector.tensor_tensor(oht[:], rb, db, op=OP.is_equal)
                ohtiles[blk] = (oht, w)
                return ohtiles[blk]

            xblk = None
            XB = 8 * 128 // TILE  # tiles per xT block (1024 dsts)
            for t in range(NT):
                lst = chunk_list[t]
                aggT = epool.tile([D, TILE], F32, tag="aggT")
                if lst:
                    pagg = ppool.tile([D, TILE], F32, tag="pagg")
                    nlst = len(lst)
                    for i, (b, pos, ch) in enumerate(lst):
                        call_i = pos // (CH // 128)
                        slot = pos % (CH // 128)
                        while issued[b] <= call_i:
                            issue_call(b)
                        oht, _w = get_oh(ch // 16)
                        nc.tensor.matmul(
                            pagg[:], btiles[(b, call_i)][:, slot, :],
                            oht[:, ch % 16, :],
                            start=(i == 0), stop=(i == nlst - 1))
                    nc.vector.tensor_copy(aggT[:], pagg[:])
                else:
                    nc.vector.memset(aggT[:], 0.0)

                if t % XB == 0:
                    w = min(XB, NT - t) * TILE
                    xblk = xpool.tile([D, w], F32, tag="xT")
                    nc.sync.dma_start(xblk[:],
                                      xTp[:, t * TILE:t * TILE + w])
                xsl = xblk[:, (t % XB) * TILE:(t % XB + 1) * TILE]

                p_h = ppool.tile([D, TILE], F32, tag="ph")
                nc.tensor.matmul(p_h[:], t_wrelT[:], aggT[:],
                                 start=True, stop=False)
                nc.tensor.matmul(p_h[:], t_wcombT[:], xsl,
                                 start=False, stop=True)
                hT = epool.tile([D, TILE], F32, tag="hT")
                nc.scalar.activation(hT[:], p_h[:], AF.Tanh,
                                     bias=t_bcomb[:], scale=1.0)
                xnT = epool.tile([D, TILE], F32, tag="xnT")
                nc.vector.scalar_tensor_tensor(
                    xnT[:], hT[:], 0.1, xsl, op0=OP.mult, op1=OP.add)
                p_o = ppool.tile([C, TILE], F32, tag="po")
                nc.tensor.matmul(p_o[:], t_wlinT[:], xnT[:],
                                 start=True, stop=True)
                ot = epool.tile([C, TILE], F32, tag="ot")
                nc.scalar.activation(ot[:], p_o[:], AF.Sigmoid,
                                     bias=t_blin[:], scale=1.0)
                nc.sync.dma_start(outT[:, t * TILE:(t + 1) * TILE], ot[:])

    nc.compile()
    return nc


def _host_prep_weights(W_rel, b_rel, W_root, W_anti, b_anti, W_lin, b_lin, D):
    aW = W_anti - W_anti.T - 0.1 * np.eye(D, dtype=np.float32)
    W_comb = W_root + aW
    wrelT = np.ascontiguousarray(W_rel.T.astype(np.float32))
    wcombT = np.ascontiguousarray(W_comb.T.astype(np.float32))
    wlinT = np.ascontiguousarray(W_lin.T.astype(np.float32))
    bcomb = (b_rel + b_anti).astype(np.float32).reshape(-1, 1)
    blin = b_lin.astype(np.float32).reshape(-1, 1)
    return wrelT, wcombT, wlinT, bcomb, blin


TRACE = False
LAST_RESULTS = None
_BUILD_CACHE = {}


def _run(inputs, cfg):
    global LAST_RESULTS
    NCORES, NPC, NPC_PAD, D = (cfg["NCORES"], cfg["NPC"], cfg["NPC_PAD"],
                               cfg["D"])

    edge_index = np.asarray(inputs["edge_index"], dtype=np.int32)
    x = np.asarray(inputs["embed_w"], dtype=np.float32)

    sched, gidx_list, dlt_list = _prep_edges(edge_index, cfg)

    key = (sched["TOT"], sched["NCH"],
           tuple(np.asarray(sched["K0"]).reshape(-1).tolist()),
           tuple(sorted(cfg.items())))
    if key not in _BUILD_CACHE:
        _BUILD_CACHE[key] = _build(cfg, sched)
    nc = _BUILD_CACHE[key]

    wrelT, wcombT, wlinT, bcomb, blin = _host_prep_weights(
        np.asarray(inputs["W_rel"], np.float32),
        np.asarray(inputs["b_rel"], np.float32),
        np.asarray(inputs["W_root"], np.float32),
        np.asarray(inputs["W_anti"], np.float32),
        np.asarray(inputs["b_anti"], np.float32),
        np.asarray(inputs["W_lin"], np.float32),
        np.asarray(inputs["b_lin"], np.float32),
        D)
    ramp = np.tile(np.arange(cfg["TILE"], dtype=np.float32), (128, 1))

    in_maps = []
    for c in range(NCORES):
        xTc = np.zeros((D, NPC_PAD), dtype=np.float32)
        xTc[:, :NPC] = x[c * NPC:(c + 1) * NPC].T
        in_maps.append({
            "xtab": x,
            "gidx": gidx_list[c],
            "dlt": dlt_list[c],
            "xT": xTc,
            "wrelT": wrelT,
            "wcombT": wcombT,
            "wlinT": wlinT,
            "bcomb": bcomb,
            "blin": blin,
            "ramp": ramp,
        })

    res = run_bass_kernel_spmd(nc, in_maps, list(range(NCORES)), trace=TRACE)
    LAST_RESULTS = res
    out = np.concatenate(
        [np.asarray(res.results[c]["outT"]).T[:NPC] for c in range(NCORES)],
        axis=0).astype(np.float32)
    return out


def kernel(**inputs) -> np.ndarray:
    return _run(inputs, _cfg_full())
